# revision 16
# baseline (speedup 1.0000x reference)
"""Trainium2 Bass kernel for nn_LossFunction_40346922778857.

Computes: scatter-loss over x (256,128,768).
  x1 = x[::2], x2 = x[1::2]  (each (128,128,768))
  per half: within (D,D), between (D,D) scatter matrices, corr-normalized,
  loss = sum((w1-w2)^2) + sum((b1-b2)^2).

Strategy (data-parallel over b across 8 cores):
  within = (G - N * Xbar^T Xbar) / (B*N)   with G = X^T X over (B*N, D)
  between = N * (Xbar^T Xbar - B mean mean^T) / (B*N)
  Each core computes partial G (upper-triangle 128-row blocks only; fp8e4
  inputs with DoubleRow 2x tensor-engine packing, fp32 PSUM accumulation)
  for its 16 even + 16 odd b's.  Host sums the 8 partial results, adds the
  fp32-exact row-sum terms, and finishes the O(D^2) algebra in float64.

  Perf design (measured ~34us HW exec vs 43.6us baseline):
  - matmul chunks all >=256 cols so the 107ns DoubleRow LDWEIGHTS stays
    hidden (every chunk <=512 cols, the PSUM bank width);
  - input packed partition-major (1.5-3KB DMA descriptors) and FLOODED
    on the sync HWDGE ring in consumption order: deep per-queue backlog
    sustains ~300-477GB/s, while paced/chained DMAs expose ~100ns/desc
    gaps and crawl; small leading pieces start compute ~9.4us;
  - the first sweep of each half is k-outer over 3 row blocks so each
    arrived 256-row piece unlocks ~0.9us of PE work -- a starved PE
    drops into the HAM clock gate and runs ~30% slow for microseconds;
  - per-half outputs packed into a flat [128, 2688] buffer and shipped
    with 3 DMAs instead of 6 (descriptor generation costs ~0.6us per
    DMA, serialized per engine); the last tiny group rides the by-then
    idle sync ring.
  Fixed framework overhead (~6us head + ~8us semaphore-reset tail) and
  the 19.9us MAC-roofline compute bound the remaining time.
"""

import numpy as np

P = 128          # partitions / rows per b
D = 768          # feature dim
NB = 16          # number of b's (tiles) per half per core
NCORES = 8
NBLK = D // P    # 6 row blocks of G
ND = NB // 2     # 256-row pieces per half per core (DoubleRow contraction)

WIDTHS = [D - P * i for i in range(NBLK)]            # 768,640,512,384,256,128
OFFS = np.concatenate([[0], np.cumsum(WIDTHS)])      # output packing offsets
OW = int(OFFS[-1])                                   # 2688
OGROUPS = [(0, 1, 2), (3, 4), (5,)]                  # merged output DMAs

_STATE = {}
LAST = {}


def _chunks_for(w_all):
    """Matmul chunking: every chunk >=256 cols (so the 107ns DoubleRow
    LDWEIGHTS stays hidden) and <=512 (PSUM bank width)."""
    if w_all <= 512:
        return [(0, w_all)]
    h = w_all // 2
    return [(0, h), (h, w_all - h)]


def _build():
    import concourse.tile as tile
    from concourse import bacc, mybir
    from concourse.tile import add_dep_helper

    nc = bacc.Bacc("TRN2", target_bir_lowering=False, debug=False,
                   num_devices=NCORES)

    in_dt = mybir.dt.float8e4
    # partition-major packing: per partition p, piece td's 2x768 bytes are
    # contiguous, so each DMA descriptor is one 1.5-3KB run per partition.
    xins = [nc.dram_tensor(f"x{h}", [P, ND, 2, D], in_dt,
                           kind="ExternalInput").ap() for h in range(2)]
    outs = [nc.dram_tensor(f"o{h}", [P, OW], mybir.dt.bfloat16,
                           kind="ExternalOutput").ap() for h in range(2)]

    with tile.TileContext(nc) as tc:
        with tc.tile_pool(name="xp", bufs=11) as xp, \
             tc.tile_pool(name="pp", bufs=6, space="PSUM") as pp, \
             tc.tile_pool(name="wpp", bufs=1, space="PSUM") as wpp, \
             tc.tile_pool(name="op", bufs=6) as op:
            # PE warm-up: dummy matmuls while input DMAs stream, so the HAM
            # clock gate is at 8/8 when real matmuls start.
            wt = xp.tile([P, P], mybir.dt.float16, tag="wt")
            nc.vector.memset(wt[:], 0.0)
            wps = wpp.tile([P, P], mybir.dt.float32, tag="wps")
            for _ in range(23):
                nc.tensor.matmul(wps[:], wt[:], wt[:], start=True, stop=True)

            # Input stream: everything floods one ring (deep per-queue
            # backlog sustains ~477GB/s; paced/chained DMAs expose a
            # ~100ns/descriptor gap and crawl).  Strict per-queue FIFO
            # keeps arrival in submission = consumption order; the two
            # leading 256-row pieces are small so compute starts earliest.
            h0_tiles = [xp.tile([P, 2, D], in_dt, tag="xt", name=f"x0t{td}")
                        for td in range(4)] + \
                       [xp.tile([P, 2, 2, D], in_dt, tag="xt", name=f"x0q{q}")
                        for q in (2, 3)]
            h1_tiles = [xp.tile([P, 2, 2, D], in_dt, tag="xt", name=f"x1q{q}")
                        for q in range(4)]
            for td in range(4):
                nc.sync.dma_start(out=h0_tiles[td][:], in_=xins[0][:, td])
            for q in (2, 3):
                nc.sync.dma_start(out=h0_tiles[2 + q][:],
                                  in_=xins[0][:, 2 * q:2 * q + 2])
            for q in range(4):
                d = nc.sync.dma_start(out=h1_tiles[q][:],
                                      in_=xins[1][:, 2 * q:2 * q + 2])
            d_last_in = d

            def lhs_rhs(h, td, c0, off, w):
                if h == 0:
                    xt = (h0_tiles[td][:] if td < 4 else
                          h0_tiles[2 + td // 2][:, td % 2])
                else:
                    xt = h1_tiles[td // 2][:, td % 2]
                return xt[:, :, c0:c0 + P], xt[:, :, c0 + off:c0 + off + w]

            # First sweep of each half is k-outer across 3 row-blocks so one
            # arrived piece unlocks ~0.9us of PE work (no input starvation);
            # later per-block sweeps retire PSUM + outputs sooner.
            for h in range(2):
                ots = {}
                for g, grp in enumerate(OGROUPS):
                    w_g = int(OFFS[grp[-1] + 1] - OFFS[grp[0]])
                    ots[g] = op.tile([P, w_g], mybir.dt.bfloat16, tag="ot",
                                     name=f"o{h}g{g}")
                for sweep in ((0, 1, 2), (3,), (4,), (5,)):
                    pts = {}
                    for i in sweep:
                        for ci in range(len(_chunks_for(WIDTHS[i]))):
                            pts[i, ci] = pp.tile([P, 512], mybir.dt.float32,
                                                 tag="ps", name=f"ps{h}b{i}c{ci}")
                    for td in range(ND):
                        for i in sweep:
                            c0 = P * i
                            for ci, (off, w) in enumerate(_chunks_for(WIDTHS[i])):
                                lhsT, rhs = lhs_rhs(h, td, c0, off, w)
                                nc.tensor.matmul(
                                    pts[i, ci][:, :w], lhsT, rhs,
                                    start=(td == 0), stop=(td == ND - 1),
                                    perf_mode=mybir.MatmulPerfMode.DoubleRow)
                    for i in sweep:
                        g = 0 if i < 3 else (1 if i < 5 else 2)
                        base = int(OFFS[i] - OFFS[OGROUPS[g][0]])
                        for ci, (off, w) in enumerate(_chunks_for(WIDTHS[i])):
                            nc.vector.tensor_copy(
                                ots[g][:, base + off:base + off + w],
                                pts[i, ci][:, :w])
                for g, grp in enumerate(OGROUPS):
                    o0 = int(OFFS[grp[0]])
                    o1 = int(OFFS[grp[-1] + 1])
                    # scalar-engine HWDGE ring, gated behind the input
                    # stream so output traffic never steals input BW; the
                    # last (tiny) group goes on the by-then-idle sync ring
                    # so its descriptor gen isn't queued behind group 1's.
                    eng = nc.sync if g == len(OGROUPS) - 1 else nc.scalar
                    dout = eng.dma_start(out=outs[h][:, o0:o1], in_=ots[g][:])
                    add_dep_helper(dout.ins, d_last_in.ins,
                                   reason="outputs after inputs")
    nc.compile()
    return nc


def _get_nc():
    if "nc" not in _STATE:
        _STATE["nc"] = _build()
    return _STATE["nc"]


def _prep_half(xh):
    """xh: (128, 128, 768) f32 for one half -> per-core (P, ND, 2, D) fp8."""
    import ml_dtypes
    out = []
    for c in range(NCORES):
        blk = xh[NB * c:NB * (c + 1)].astype(ml_dtypes.float8_e4m3)
        # b = 2*td + j -> (p, td, j, f): partition-major
        out.append(np.ascontiguousarray(
            blk.reshape(ND, 2, P, D).transpose(2, 0, 1, 3)))
    return out


def kernel(x, label=None, genre_label=None, _trace=False):
    from concourse.bass_utils import run_bass_kernel_spmd

    nc = _get_nc()

    x = np.asarray(x, dtype=np.float32)
    halves = [_prep_half(x[0::2]), _prep_half(x[1::2])]
    in_maps = [{"x0": halves[0][c], "x1": halves[1][c]} for c in range(NCORES)]

    # First execution of a freshly compiled NEFF has been observed to be
    # flaky (garbage output or device error); validate and retry.
    res = None
    for attempt in range(3):
        try:
            res = run_bass_kernel_spmd(nc, in_maps, list(range(NCORES)),
                                       trace=_trace)
        except Exception:
            if attempt == 2:
                raise
            continue
        ok = all(
            np.isfinite(np.asarray(res.results[c][f"o{h}"],
                                   dtype=np.float32)).all()
            and np.any(np.asarray(res.results[c][f"o{h}"], dtype=np.float32))
            for c in range(NCORES) for h in range(2))
        if ok:
            break
    LAST["res"] = res

    B = x.shape[0] // 2          # 128 b's per half
    N = x.shape[1]               # 128 rows per b
    tol = B * N

    loss = 0.0
    for h in range(2):
        xh = x[h::2]                                   # (B, N, D) fp32
        G = np.zeros((D, D), dtype=np.float64)
        for c in range(NCORES):
            o = np.asarray(res.results[c][f"o{h}"], dtype=np.float64)
            for i in range(NBLK):
                blk = o[:, OFFS[i]:OFFS[i + 1]]        # (128, 768-128i)
                G[P * i:P * (i + 1), P * i:D] += blk
        for i in range(NBLK):
            for j in range(i + 1, NBLK):
                ri = slice(P * i, P * (i + 1))
                rj = slice(P * j, P * (j + 1))
                G[rj, ri] = G[ri, rj].T
        # row-sum terms from fp32 input (exact, cheap on host)
        S = xh.sum(axis=1, dtype=np.float64)           # (B, D)
        xbar = S / N
        M = xbar.T @ xbar
        mean = xbar.mean(axis=0)
        within = (G - N * M) / tol
        between = N * (M - B * np.outer(mean, mean)) / tol
        w_h = within / np.sqrt(np.sum(np.diagonal(within) ** 2))
        b_h = between / np.sqrt(np.sum(np.diagonal(between) ** 2))
        if h == 0:
            w0, b0 = w_h, b_h
        else:
            loss = np.sum((w0 - w_h) ** 2) + np.sum((b0 - b_h) ** 2)
    return np.asarray(loss, dtype=np.float32)


# revision 17
# speedup vs baseline: 1.0701x; 1.0701x over previous
"""Trainium2 Bass kernel for nn_LossFunction_40346922778857.

Computes: scatter-loss over x (256,128,768).
  x1 = x[::2], x2 = x[1::2]  (each (128,128,768))
  per half: within (D,D), between (D,D) scatter matrices, corr-normalized,
  loss = sum((w1-w2)^2) + sum((b1-b2)^2).

Strategy (data-parallel over b across 8 cores):
  within = (G - N * Xbar^T Xbar) / (B*N)   with G = X^T X over (B*N, D)
  between = N * (Xbar^T Xbar - B mean mean^T) / (B*N)
  Each core computes partial G (upper-triangle 128-row blocks only; fp8e4
  inputs with DoubleRow 2x tensor-engine packing, fp32 PSUM accumulation)
  for its 16 even + 16 odd b's.  Host sums the 8 partial results, adds the
  fp32-exact row-sum terms, and finishes the O(D^2) algebra in float64.

  Perf design (measured ~34us HW exec vs 43.6us baseline):
  - matmul chunks all >=256 cols so the 107ns DoubleRow LDWEIGHTS stays
    hidden (every chunk <=512 cols, the PSUM bank width);
  - input packed partition-major (1.5-3KB DMA descriptors) and FLOODED
    on the sync HWDGE ring in consumption order: deep per-queue backlog
    sustains ~300-477GB/s, while paced/chained DMAs expose ~100ns/desc
    gaps and crawl; small leading pieces start compute ~9.4us;
  - the first sweep of each half is k-outer over 3 row blocks so each
    arrived 256-row piece unlocks ~0.9us of PE work -- a starved PE
    drops into the HAM clock gate and runs ~30% slow for microseconds;
  - per-half outputs packed into a flat [128, 2688] buffer and shipped
    with 3 DMAs instead of 6 (descriptor generation costs ~0.6us per
    DMA, serialized per engine); the last tiny group rides the by-then
    idle sync ring.
  Fixed framework overhead (~6us head + ~8us semaphore-reset tail) and
  the 19.9us MAC-roofline compute bound the remaining time.
"""

import numpy as np

P = 128          # partitions / rows per b
D = 768          # feature dim
NB = 16          # number of b's (tiles) per half per core
NCORES = 8
NBLK = D // P    # 6 row blocks of G
ND = NB // 2     # 256-row pieces per half per core (DoubleRow contraction)

WIDTHS = [D - P * i for i in range(NBLK)]            # 768,640,512,384,256,128
OFFS = np.concatenate([[0], np.cumsum(WIDTHS)])      # output packing offsets
OW = int(OFFS[-1])                                   # 2688
OGROUPS = [(0, 1, 2), (3, 4), (5,)]                  # merged output DMAs

_STATE = {}
LAST = {}


def _chunks_for(w_all):
    """Matmul chunking: every chunk >=256 cols (so the 107ns DoubleRow
    LDWEIGHTS stays hidden) and <=512 (PSUM bank width)."""
    if w_all <= 512:
        return [(0, w_all)]
    h = w_all // 2
    return [(0, h), (h, w_all - h)]


def _build():
    import concourse.tile as tile
    from concourse import bacc, mybir
    from concourse.tile import add_dep_helper

    nc = bacc.Bacc("TRN2", target_bir_lowering=False, debug=False,
                   num_devices=NCORES)

    in_dt = mybir.dt.float8e4
    # partition-major packing: per partition p, piece td's 2x768 bytes are
    # contiguous, so each DMA descriptor is one 1.5-3KB run per partition.
    xins = [nc.dram_tensor(f"x{h}", [P, ND, 2, D], in_dt,
                           kind="ExternalInput").ap() for h in range(2)]
    outs = [nc.dram_tensor(f"o{h}", [P, OW], mybir.dt.bfloat16,
                           kind="ExternalOutput").ap() for h in range(2)]

    with tile.TileContext(nc) as tc:
        with tc.tile_pool(name="xp", bufs=11) as xp, \
             tc.tile_pool(name="pp", bufs=6, space="PSUM") as pp, \
             tc.tile_pool(name="wpp", bufs=1, space="PSUM") as wpp, \
             tc.tile_pool(name="op", bufs=6) as op:
            # PE warm-up: dummy matmuls while input DMAs stream, so the HAM
            # clock gate is at 8/8 when real matmuls start.
            wt = xp.tile([P, P], mybir.dt.float16, tag="wt")
            nc.vector.memset(wt[:], 0.0)
            wps = wpp.tile([P, P], mybir.dt.float32, tag="wps")
            for _ in range(23):
                nc.tensor.matmul(wps[:], wt[:], wt[:], start=True, stop=True)

            # Input stream: everything floods one ring (deep per-queue
            # backlog sustains ~477GB/s; paced/chained DMAs expose a
            # ~100ns/descriptor gap and crawl).  Strict per-queue FIFO
            # keeps arrival in submission = consumption order; the two
            # leading 256-row pieces are small so compute starts earliest.
            h0_tiles = [xp.tile([P, 2, D], in_dt, tag="xt", name=f"x0t{td}")
                        for td in range(4)] + \
                       [xp.tile([P, 2, 2, D], in_dt, tag="xt", name=f"x0q{q}")
                        for q in (2, 3)]
            h1_tiles = [xp.tile([P, 2, 2, D], in_dt, tag="xt", name=f"x1q{q}")
                        for q in range(4)]
            for td in range(4):
                nc.sync.dma_start(out=h0_tiles[td][:], in_=xins[0][:, td])
            for q in (2, 3):
                nc.sync.dma_start(out=h0_tiles[2 + q][:],
                                  in_=xins[0][:, 2 * q:2 * q + 2])
            for q in range(4):
                d = nc.sync.dma_start(out=h1_tiles[q][:],
                                      in_=xins[1][:, 2 * q:2 * q + 2])
            d_last_in = d

            def lhs_rhs(h, td, c0, off, w):
                if h == 0:
                    xt = (h0_tiles[td][:] if td < 4 else
                          h0_tiles[2 + td // 2][:, td % 2])
                else:
                    xt = h1_tiles[td // 2][:, td % 2]
                return xt[:, :, c0:c0 + P], xt[:, :, c0 + off:c0 + off + w]

            # First sweep of each half is k-outer across 3 row-blocks so one
            # arrived piece unlocks ~0.9us of PE work (no input starvation);
            # later per-block sweeps retire PSUM + outputs sooner.
            for h in range(2):
                ots = {}
                for g, grp in enumerate(OGROUPS):
                    w_g = int(OFFS[grp[-1] + 1] - OFFS[grp[0]])
                    ots[g] = op.tile([P, w_g], mybir.dt.bfloat16, tag="ot",
                                     name=f"o{h}g{g}")
                for sweep in ((0, 1, 2), (3,), (4,), (5,)):
                    pts = {}
                    for i in sweep:
                        for ci in range(len(_chunks_for(WIDTHS[i]))):
                            pts[i, ci] = pp.tile([P, 512], mybir.dt.float32,
                                                 tag="ps", name=f"ps{h}b{i}c{ci}")
                    for td in range(ND):
                        for i in sweep:
                            c0 = P * i
                            for ci, (off, w) in enumerate(_chunks_for(WIDTHS[i])):
                                lhsT, rhs = lhs_rhs(h, td, c0, off, w)
                                nc.tensor.matmul(
                                    pts[i, ci][:, :w], lhsT, rhs,
                                    start=(td == 0), stop=(td == ND - 1),
                                    perf_mode=mybir.MatmulPerfMode.DoubleRow)
                    for i in sweep:
                        g = 0 if i < 3 else (1 if i < 5 else 2)
                        base = int(OFFS[i] - OFFS[OGROUPS[g][0]])
                        for ci, (off, w) in enumerate(_chunks_for(WIDTHS[i])):
                            nc.vector.tensor_copy(
                                ots[g][:, base + off:base + off + w],
                                pts[i, ci][:, :w])
                for g, grp in enumerate(OGROUPS):
                    o0 = int(OFFS[grp[0]])
                    o1 = int(OFFS[grp[-1] + 1])
                    # Gated behind the input stream so output traffic never
                    # steals input BW.  Group 0 retires mid-compute (not
                    # critical); the two trailing groups are split by
                    # partition halves across the sync+scalar rings so
                    # their descriptor gen and per-queue transfer pacing
                    # run in parallel right after the last matmuls.
                    if g == 0:
                        douts = [nc.scalar.dma_start(
                            out=outs[h][:, o0:o1], in_=ots[g][:])]
                    else:
                        douts = [
                            nc.sync.dma_start(out=outs[h][0:64, o0:o1],
                                              in_=ots[g][0:64]),
                            nc.scalar.dma_start(out=outs[h][64:128, o0:o1],
                                                in_=ots[g][64:128]),
                        ]
                    for dout in douts:
                        add_dep_helper(dout.ins, d_last_in.ins,
                                       reason="outputs after inputs")
    nc.compile()
    return nc


def _get_nc():
    if "nc" not in _STATE:
        _STATE["nc"] = _build()
    return _STATE["nc"]


def _prep_half(xh):
    """xh: (128, 128, 768) f32 for one half -> per-core (P, ND, 2, D) fp8."""
    import ml_dtypes
    out = []
    for c in range(NCORES):
        blk = xh[NB * c:NB * (c + 1)].astype(ml_dtypes.float8_e4m3)
        # b = 2*td + j -> (p, td, j, f): partition-major
        out.append(np.ascontiguousarray(
            blk.reshape(ND, 2, P, D).transpose(2, 0, 1, 3)))
    return out


def kernel(x, label=None, genre_label=None, _trace=False):
    from concourse.bass_utils import run_bass_kernel_spmd

    nc = _get_nc()

    x = np.asarray(x, dtype=np.float32)
    halves = [_prep_half(x[0::2]), _prep_half(x[1::2])]
    in_maps = [{"x0": halves[0][c], "x1": halves[1][c]} for c in range(NCORES)]

    # First execution of a freshly compiled NEFF has been observed to be
    # flaky (garbage output or device error); validate and retry.
    res = None
    for attempt in range(3):
        try:
            res = run_bass_kernel_spmd(nc, in_maps, list(range(NCORES)),
                                       trace=_trace)
        except Exception:
            if attempt == 2:
                raise
            continue
        ok = all(
            np.isfinite(np.asarray(res.results[c][f"o{h}"],
                                   dtype=np.float32)).all()
            and np.any(np.asarray(res.results[c][f"o{h}"], dtype=np.float32))
            for c in range(NCORES) for h in range(2))
        if ok:
            break
    LAST["res"] = res

    B = x.shape[0] // 2          # 128 b's per half
    N = x.shape[1]               # 128 rows per b
    tol = B * N

    loss = 0.0
    for h in range(2):
        xh = x[h::2]                                   # (B, N, D) fp32
        G = np.zeros((D, D), dtype=np.float64)
        for c in range(NCORES):
            o = np.asarray(res.results[c][f"o{h}"], dtype=np.float64)
            for i in range(NBLK):
                blk = o[:, OFFS[i]:OFFS[i + 1]]        # (128, 768-128i)
                G[P * i:P * (i + 1), P * i:D] += blk
        for i in range(NBLK):
            for j in range(i + 1, NBLK):
                ri = slice(P * i, P * (i + 1))
                rj = slice(P * j, P * (j + 1))
                G[rj, ri] = G[ri, rj].T
        # row-sum terms from fp32 input (exact, cheap on host)
        S = xh.sum(axis=1, dtype=np.float64)           # (B, D)
        xbar = S / N
        M = xbar.T @ xbar
        mean = xbar.mean(axis=0)
        within = (G - N * M) / tol
        between = N * (M - B * np.outer(mean, mean)) / tol
        w_h = within / np.sqrt(np.sum(np.diagonal(within) ** 2))
        b_h = between / np.sqrt(np.sum(np.diagonal(between) ** 2))
        if h == 0:
            w0, b0 = w_h, b_h
        else:
            loss = np.sum((w0 - w_h) ** 2) + np.sum((b0 - b_h) ** 2)
    return np.asarray(loss, dtype=np.float32)
